# revision 21
# baseline (speedup 1.0000x reference)
"""Trainium2 Bass kernel for nn_DialogueGCNModel (DialogueGCN forward).

Strategy (data-parallel over dialogues, 4 dialogues per core):
  - Edges never cross dialogues (windowed construction), so the RGCN
    scatter/gather is reformulated as dense per-dialogue banded-adjacency
    matmuls: agg^T = (sum_r xr_r^T @ A_r^T) * (1/deg), with exact 0/1
    adjacency masks shipped as fp8 and the degree scaling applied in f32.
  - Everything on-device is dense PE matmuls (bf16/fp8 in, f32 accumulate),
    softmax/log-softmax in f32 on ACT/DVE.
  - Host does index preprocessing only: shard x, build per-dialogue 0/1
    adjacency masks from the edge lists, transpose/pack layouts, cast.
  - Emission is stage-major across the 4 dialogues so the PE never stalls
    on one dialogue's softmax chain; activations are function-major to
    avoid ACT LUT-table reloads; inputs move as a few large multi-dim-AP
    DMAs ordered by first use, and the PE runs dependency-free warm-up
    matmuls during the DMA lead-in to hold the HAM clock at 2.4 GHz.

kernel(**inputs) takes FULL inputs, runs 8-core SPMD via
bass_utils.run_bass_kernel_spmd, returns the FULL (8192, 7) f32 output.
"""

import numpy as np
import ml_dtypes

BF16 = ml_dtypes.bfloat16
FP8 = ml_dtypes.float8_e4m3

# Problem constants (hardcoded per contract)
B, L, D, H, R, NB, C = 32, 256, 1024, 128, 8, 30, 7
MEM = D + H            # 1152
N = B * L              # 8192
NCORES = 8
DPC = B // NCORES      # dialogues per core = 4
NLOC = DPC * L         # nodes per core = 1024
NT = NLOC // 128       # node tiles per core = 8
KT = D // 128          # contraction tiles over D = 8
MT = MEM // 128        # tiles over MEM = 9

_cache = {}


def _build_program(use_mask, halves, ablocks, stop_after=None):
    import concourse.bacc as bacc
    import concourse.tile as tile
    import concourse.mybir as mybir
    import concourse.bass as bass
    from concourse.masks import make_identity

    dt = mybir.dt
    f32, bf16, fp8 = dt.float32, dt.bfloat16, dt.float8e4
    AX = mybir.AxisListType.X
    AF = mybir.ActivationFunctionType
    OP = mybir.AluOpType

    nc = bacc.Bacc("TRN2", target_bir_lowering=False, debug=False,
                   num_devices=NCORES)

    dram = nc.dram_tensor
    xt_d = dram("xt", [D, NLOC], bf16, kind="ExternalInput")        # x^T [d, n]
    xv_d = dram("xv", [NLOC, D], bf16, kind="ExternalInput")        # x [n, d]
    wrel_d = dram("wrel", [D, R * H], bf16, kind="ExternalInput")   # [d, r*H+h]
    wr1_d = dram("wr1", [D, H], bf16, kind="ExternalInput")
    at_d = dram("at", [DPC, R, L, L], fp8, kind="ExternalInput")    # A^T (0/1)
    bt_d = dram("bt", [DPC, L, L], fp8, kind="ExternalInput")       # B^T (0/1)
    invd_d = dram("invd", [DPC, L], f32, kind="ExternalInput")      # 1/deg
    w2_d = dram("w2", [2, H, H], bf16, kind="ExternalInput")        # rel2, root2
    wt_d = dram("wt", [MEM, MEM], bf16, kind="ExternalInput")
    wlin_d = dram("wlin", [MEM, H], bf16, kind="ExternalInput")
    wfc_d = dram("wfc", [H, C], bf16, kind="ExternalInput")
    bias_d = dram("bias", [128, 12], f32, kind="ExternalInput")
    bfc_d = dram("bfc", [1, C], bf16, kind="ExternalInput")
    if use_mask:
        um_d = dram("um", [DPC, 2, L], f32, kind="ExternalInput")   # um2, um
    out_d = dram("out", [NLOC, C], f32, kind="ExternalOutput")

    with tile.TileContext(nc) as tc:
        from contextlib import ExitStack
        with ExitStack() as ctx:
            consts = ctx.enter_context(tc.tile_pool(name="consts", bufs=1))
            big = ctx.enter_context(tc.tile_pool(name="big", bufs=1))
            work = ctx.enter_context(tc.tile_pool(name="work", bufs=4))
            ps = ctx.enter_context(tc.tile_pool(name="ps", bufs=5, space="PSUM"))
            pst = ctx.enter_context(tc.tile_pool(name="pst", bufs=2, space="PSUM"))

            dma_a = nc.sync.dma_start      # queue A: PE-critical operands
            dma_b = nc.scalar.dma_start    # queue B: everything else
            mm = nc.tensor.matmul

            # ---- persistent operand loads (one DMA per tensor) ----
            xt = consts.tile([128, KT, NLOC], bf16)
            dma_a(out=xt, in_=xt_d[:].rearrange("(k p) n -> p k n", p=128))
            wrel = consts.tile([128, KT, R * H], bf16)
            for h2 in range(2):
                dma_a(out=wrel[:, :, h2 * 512:(h2 + 1) * 512],
                      in_=wrel_d[:, h2 * 512:(h2 + 1) * 512]
                      .rearrange("(k p) n -> p k n", p=128))
            wr1 = consts.tile([128, KT, H], bf16)
            dma_a(out=wr1, in_=wr1_d[:].rearrange("(k p) n -> p k n", p=128))
            wt = consts.tile([128, MT, MEM], bf16)
            dma_a(out=wt, in_=wt_d[:].rearrange("(m p) n -> p m n", p=128))
            at = consts.tile([128, DPC, R, 2, L], fp8)
            dma_a(out=at,
                  in_=at_d[:].rearrange("d r (st p) t -> p d r st t", p=128))
            bt = consts.tile([128, DPC, 2, L], fp8)
            dma_a(out=bt, in_=bt_d[:].rearrange("d (st p) t -> p d st t", p=128))
            xv = consts.tile([128, NT, D], bf16)
            dma_a(out=xv, in_=xv_d[:].rearrange("(i p) n -> p i n", p=128))
            wlin = consts.tile([128, MT, H], bf16)
            dma_a(out=wlin, in_=wlin_d[:].rearrange("(m p) n -> p m n", p=128))
            w2 = consts.tile([128, 2, H], bf16)
            dma_b(out=w2, in_=w2_d[:].rearrange("j p h -> p j h"))
            wfc = consts.tile([128, C], bf16)
            dma_b(out=wfc, in_=wfc_d[:])
            bias = consts.tile([128, 12], f32)
            dma_b(out=bias, in_=bias_d[:])
            bfc = consts.tile([1, C], bf16)
            dma_b(out=bfc, in_=bfc_d[:])
            ones_row = consts.tile([1, 128], bf16)
            nc.vector.memset(ones_row, 1.0)
            ident = consts.tile([128, 128], bf16)
            make_identity(nc, ident)
            # keep the PE busy (HAM warm) during the input-DMA lead-in;
            # `warm` psum is never read.
            warm_in = consts.tile([128, 128], bf16)
            nc.vector.memset(warm_in, 0.0)
            pw = ctx.enter_context(tc.tile_pool(name="pw", bufs=1, space="PSUM"))
            warm = pw.tile([128, 128], f32, tag="warm")
            for _ in range(200):
                mm(warm, lhsT=warm_in, rhs=warm_in, start=True, stop=True,
                   skip_group_check=True)

            def bcast(dst, src_ap):
                bc = bass.AP(tensor=src_ap.tensor, offset=src_ap.offset,
                             ap=[[0, 128]] + list(src_ap.ap))
                nc.gpsimd.dma_start(out=dst, in_=bc)

            invd = consts.tile([128, DPC, L], f32)
            bcast(invd, invd_d[:])
            if use_mask:
                um = consts.tile([128, DPC, 2, L], f32)
                bcast(um, um_d[:])

            # ---- stage 1: xr[n, (r,h)] = x @ w_rel (all relations) ----
            xr = consts.tile([128, NT, R * H], bf16)
            for h2, i in sorted(
                    (h2, i) for i in range(NT) for h2 in halves[i]):
                p = ps.tile([128, 512], f32, tag="mm")
                for k in range(KT):
                    mm(p, lhsT=xt[:, k, i * 128:(i + 1) * 128],
                       rhs=wrel[:, k, h2 * 512:(h2 + 1) * 512],
                       start=(k == 0), stop=(k == KT - 1))
                nc.vector.tensor_copy(xr[:, i, h2 * 512:(h2 + 1) * 512], p)

            if stop_after == "xr":
                return _finish(nc)

            out1T = consts.tile([128, DPC, L], bf16)   # [h, dlg, n]
            out1 = consts.tile([128, NT, H], bf16)     # [n, h]
            out2T = consts.tile([128, DPC, L], bf16)
            out2 = consts.tile([128, NT, H], bf16)
            hidT = consts.tile([128, DPC, L], bf16)

            # ---- stage 2: out1^T = (sum_r xr_r^T A_r^T)*invd + root^T + b1
            for d in range(DPC):
                n0 = d * L
                pa = ps.tile([128, 512], f32, tag="mm")
                blocks = ablocks[d]
                for bi, (r, st) in enumerate(blocks):
                    mm(pa[:, :L], lhsT=xr[:, 2 * d + st, r * H:(r + 1) * H],
                       rhs=at[:, d, r, st, :], start=(bi == 0),
                       stop=(bi == len(blocks) - 1))
                agg = work.tile([128, L], f32, tag="agg")
                nc.vector.tensor_mul(agg, pa[:, :L], invd[:, d, :])
                pr = ps.tile([128, 512], f32, tag="mm")
                for k in range(KT):
                    mm(pr[:, :L], lhsT=wr1[:, k, :], rhs=xt[:, k, n0:n0 + L],
                       start=(k == 0), stop=(k == KT - 1))
                nc.vector.scalar_tensor_tensor(
                    out=out1T[:, d, :], in0=pr[:, :L], scalar=bias[:, 0:1],
                    in1=agg, op0=OP.add, op1=OP.add)
            for d in range(DPC):
                for st in range(2):
                    tp = pst.tile([128, 128], bf16, tag="tr")
                    nc.tensor.transpose(tp, out1T[:, d, st * 128:(st + 1) * 128], ident)
                    nc.vector.tensor_copy(out1[:, 2 * d + st, :], tp)

            # ---- stage 3: GraphConv layer 2 ----
            nbTs = []
            for d in range(DPC):
                p2 = ps.tile([128, 512], f32, tag="mm")
                for st in range(2):
                    mm(p2[:, :L], lhsT=out1[:, 2 * d + st, :], rhs=bt[:, d, st, :],
                       start=(st == 0), stop=(st == 1))
                nbT = work.tile([128, L], bf16, tag="nbT")
                nc.vector.tensor_copy(nbT, p2[:, :L])
                nbTs.append(nbT)
            for d in range(DPC):
                p3 = ps.tile([128, 512], f32, tag="mm")
                mm(p3[:, :L], lhsT=w2[:, 0, :], rhs=nbTs[d], start=True, stop=False)
                mm(p3[:, :L], lhsT=w2[:, 1, :], rhs=out1T[:, d, :],
                   start=False, stop=True)
                nc.scalar.activation(out2T[:, d, :], p3[:, :L], AF.Identity,
                                     bias=bias[:, 1:2])
            for d in range(DPC):
                for st in range(2):
                    tp = pst.tile([128, 128], bf16, tag="tr")
                    nc.tensor.transpose(tp, out2T[:, d, st * 128:(st + 1) * 128], ident)
                    nc.vector.tensor_copy(out2[:, 2 * d + st, :], tp)

            if stop_after == "rgcn":
                return _finish(nc)

            # M^T / M tile accessors over MEM
            def rhs_MT(mt_i, d):
                if mt_i < KT:
                    return xt[:, mt_i, d * L:(d + 1) * L]
                return out2T[:, d, :]

            def lhsT_M(mt_i, st_i, d):
                if mt_i < KT:
                    return xv[:, 2 * d + st_i, mt_i * 128:(mt_i + 1) * 128]
                return out2[:, 2 * d + st_i, :]

            # ---- stage 5: Xc^T = w_t^T M^T + b_t ----
            XcTs = []
            for d in range(DPC):
                XcT = big.tile([128, MT, L], bf16, tag=f"XcT{d}")
                XcTs.append(XcT)
                for n2 in range(MT):
                    p4 = ps.tile([128, 512], f32, tag="mm")
                    for m in range(MT):
                        mm(p4[:, :L], lhsT=wt[:, m, n2 * 128:(n2 + 1) * 128],
                           rhs=rhs_MT(m, d), start=(m == 0), stop=(m == MT - 1))
                    nc.scalar.activation(XcT[:, n2, :], p4[:, :L], AF.Identity,
                                         bias=bias[:, 2 + n2:3 + n2])

            if stop_after == "xc":
                return _finish(nc)

            # ---- stage 6: scores -> tanh -> masked softmax -> alpha^T ----
            # function-major: all tanh, then all exp (one ACT table load each)
            zs, nmxs = {}, {}
            for d in range(DPC):
                for tt in range(2):
                    p5 = ps.tile([128, 512], f32, tag="mm")
                    for n2 in range(MT):
                        mm(p5[:, :L], lhsT=XcTs[d][:, n2, tt * 128:(tt + 1) * 128],
                           rhs=rhs_MT(n2, d), start=(n2 == 0), stop=(n2 == MT - 1))
                    z = big.tile([128, L], f32, tag=f"z{d}{tt}")
                    if use_mask:
                        nc.vector.tensor_mul(z, p5[:, :L], um[:, d, 0, :])
                        nc.scalar.activation(z, z, AF.Tanh)
                    else:
                        nc.scalar.activation(z, p5[:, :L], AF.Tanh)
                    nmx = work.tile([128, 1], f32, tag="nmx")
                    nc.vector.reduce_max(out=nmx, in_=z, axis=AX, negate=True)
                    zs[(d, tt)] = z
                    nmxs[(d, tt)] = nmx
            alfs = {}
            for d in range(DPC):
                for tt in range(2):
                    z, nmx = zs[(d, tt)], nmxs[(d, tt)]
                    ssum = work.tile([128, 1], f32, tag="ssum")
                    nc.scalar.activation(z, z, AF.Exp, bias=nmx, accum_out=ssum)
                    if use_mask:
                        nc.vector.tensor_mul(z, z, um[:, d, 1, :])
                        nc.vector.reduce_sum(out=ssum, in_=z, axis=AX)
                    rinv = work.tile([128, 1], f32, tag="rinv")
                    nc.vector.reciprocal(rinv, ssum)
                    alf = big.tile([128, L], bf16, tag=f"alf{d}{tt}")
                    nc.vector.tensor_scalar_mul(alf, z, rinv)
                    alfs[(d, tt)] = alf
            alphaTs = []
            for d in range(DPC):
                alphaT = big.tile([128, 2, L], bf16, tag=f"alphaT{d}")
                alphaTs.append(alphaT)
                for tt in range(2):
                    for st in range(2):
                        tp = pst.tile([128, 128], bf16, tag="tr")
                        nc.tensor.transpose(
                            tp, alfs[(d, tt)][:, st * 128:(st + 1) * 128], ident)
                        nc.vector.tensor_copy(
                            alphaT[:, st, tt * 128:(tt + 1) * 128], tp)

            if stop_after == "scores":
                return _finish(nc)

            # ---- stage 7: att^T = M^T alpha^T ; hidden^T = relu(w_lin^T att^T)
            attTs = []
            for d in range(DPC):
                attT = big.tile([128, MT, L], bf16, tag=f"attT{d}")
                attTs.append(attT)
                for m in range(MT):
                    p6 = ps.tile([128, 512], f32, tag="mm")
                    for st in range(2):
                        mm(p6[:, :L], lhsT=lhsT_M(m, st, d), rhs=alphaTs[d][:, st, :],
                           start=(st == 0), stop=(st == 1))
                    if m % 2 == 0:
                        nc.vector.tensor_copy(attT[:, m, :], p6[:, :L])
                    else:
                        nc.scalar.copy(attT[:, m, :], p6[:, :L])
            for d in range(DPC):
                p7 = ps.tile([128, 512], f32, tag="mm")
                for m in range(MT):
                    mm(p7[:, :L], lhsT=wlin[:, m, :], rhs=attTs[d][:, m, :],
                       start=(m == 0), stop=(m == MT - 1))
                nc.scalar.activation(hidT[:, d, :], p7[:, :L], AF.Relu,
                                     bias=bias[:, 11:12])

            if stop_after == "att":
                return _finish(nc)

            # ---- stage 8: logits + log_softmax (function-major) ----
            o_all = consts.tile([128, DPC, 2, 8], f32)
            nm7s, s7s = {}, {}
            for d in range(DPC):
                for tt in range(2):
                    p8 = ps.tile([128, 512], f32, tag="mm")
                    mm(p8[:, :C], lhsT=hidT[:, d, tt * 128:(tt + 1) * 128],
                       rhs=wfc, start=True, stop=False)
                    mm(p8[:, :C], lhsT=ones_row, rhs=bfc, start=False, stop=True)
                    nm7 = work.tile([128, 1], f32, tag=f"nm7_{d}{tt}")
                    nc.vector.reduce_max(out=nm7, in_=p8[:, :C], axis=AX, negate=True)
                    e7 = work.tile([128, 8], f32, tag="e7")
                    s7 = work.tile([128, 1], f32, tag=f"s7_{d}{tt}")
                    nc.scalar.activation(e7[:, :C], p8[:, :C], AF.Exp,
                                         bias=nm7, accum_out=s7)
                    nc.vector.tensor_scalar_add(o_all[:, d, tt, :C], p8[:, :C], nm7)
                    nm7s[(d, tt)], s7s[(d, tt)] = nm7, s7
            for d in range(DPC):
                for tt in range(2):
                    nm7, s7 = nm7s[(d, tt)], s7s[(d, tt)]
                    ls7 = work.tile([128, 1], f32, tag="ls7")
                    nc.scalar.activation(ls7, s7, AF.Ln)
                    nc.vector.tensor_scalar(
                        out=o_all[:, d, tt, :C], in0=o_all[:, d, tt, :C],
                        scalar1=ls7, scalar2=None, op0=OP.subtract)
            dma_a(out=out_d[:].rearrange("(d tt p) c -> p d tt c", d=DPC, tt=2),
                  in_=o_all[:, :, :, 0:C])

    return _finish(nc)


def _finish(nc):
    nc.compile()
    return nc


def prep_inputs(x, edge_src, edge_dst, edge_type, umask, basis, comp,
                w_root1, b1, w_rel2, b_rel2, w_root2, w_t, b_t,
                w_lin, b_lin, w_fc, b_fc):
    """Host-side sharding / layout prep.

    Returns (in_maps, use_mask, halves, ablocks, perm).
    Nodes are permuted within each dialogue so same-speaker nodes are
    contiguous; then each 128-node tile only needs the relation-half
    matching its speaker(s), and all-zero adjacency blocks are skipped.
    """
    x = np.asarray(x, np.float32)
    src = np.asarray(edge_src, np.int64)
    dst = np.asarray(edge_dst, np.int64)
    ety = np.asarray(edge_type, np.int64)
    umask = np.asarray(umask, np.float32)
    basis = np.asarray(basis, np.float32)
    comp = np.asarray(comp, np.float32)

    # dialogue-locality of edges (guaranteed by the windowed construction)
    g_s = src // L
    assert np.array_equal(g_s, dst // L), "edges must stay within a dialogue"

    # infer per-node speaker from edge types (etype = s_src*4 + s_dst*2 + dir);
    # fall back to identity permutation if inconsistent.
    spk_s = ety // 4
    spk_d = (ety // 2) % 2
    spk = -np.ones(N, np.int64)
    spk[src] = spk_s
    ok = bool(np.all(spk[src] == spk_s)) and bool(np.all(spk[dst] == spk_d)) \
        and bool(np.all(spk >= 0))
    if ok:
        perm = np.empty(N, np.int64)
        for g in range(B):
            sl = slice(g * L, (g + 1) * L)
            perm[sl] = g * L + np.argsort(spk[sl], kind="stable")
    else:
        perm = np.arange(N, dtype=np.int64)
    inv = np.empty(N, np.int64)
    inv[perm] = np.arange(N)

    x = x[perm]
    src = inv[src]
    dst = inv[dst]
    umask = umask.reshape(N)[perm].reshape(B, L)
    spk_p = spk[perm] if ok else spk

    # w_rel[r] = sum_b comp[r,b] basis[b]  -> layout [d, r*H+h]
    w_rel = np.einsum('rb,bdh->rdh', comp, basis)
    wrel_layout = np.ascontiguousarray(
        w_rel.transpose(1, 0, 2).reshape(D, R * H)).astype(BF16)

    deg = np.bincount(dst, minlength=N).astype(np.float64)
    inv_deg = np.where(deg > 0, 1.0 / np.maximum(deg, 1), 0.0).astype(np.float32)

    g_s = src // L
    at_all = np.zeros((B, R, L, L), np.float32)   # [dlg, r, src, dst] 0/1
    ls, ld = src % L, dst % L
    np.add.at(at_all, (g_s, ety, ls, ld), 1.0)
    bt_all = np.zeros((B, L, L), np.float32)
    np.add.at(bt_all, (g_s, ls, ld), 1.0)

    use_mask = not bool(np.all(umask == 1.0))

    bias_pack = np.zeros((128, 12), np.float32)
    bias_pack[:, 0] = np.asarray(b1, np.float32)
    bias_pack[:, 1] = np.asarray(b_rel2, np.float32)
    bias_pack[:, 2:11] = np.asarray(b_t, np.float32).reshape(9, 128).T
    bias_pack[:, 11] = np.asarray(b_lin, np.float32)

    shared = {
        "wrel": wrel_layout,
        "wr1": np.asarray(w_root1, np.float32).astype(BF16),
        "w2": np.stack([np.asarray(w_rel2, np.float32),
                        np.asarray(w_root2, np.float32)]).astype(BF16),
        "wt": np.asarray(w_t, np.float32).astype(BF16),
        "wlin": np.asarray(w_lin, np.float32).astype(BF16),
        "wfc": np.asarray(w_fc, np.float32).astype(BF16),
        "bias": bias_pack,
        "bfc": np.asarray(b_fc, np.float32).reshape(1, C).astype(BF16),
    }

    # per-core tile structure: which relation-halves each node-tile needs,
    # and which (r, st) adjacency blocks are nonzero per dialogue.
    # NOTE: the program structure must be IDENTICAL across cores (one SPMD
    # NEFF), so take the union over cores per (tile, dialogue) position.
    halves = []
    for i in range(NT):
        hs = set()
        for c in range(NCORES):
            nodes = slice(c * NLOC + i * 128, c * NLOC + (i + 1) * 128)
            if ok:
                hs |= set(np.unique(spk_p[nodes]).tolist())
            else:
                hs |= {0, 1}
        halves.append(tuple(sorted(hs)))
    ablocks = []
    for d in range(DPC):
        blk = []
        for r in range(R):
            for st in range(2):
                nz = False
                for c in range(NCORES):
                    g = c * DPC + d
                    if at_all[g, r, st * 128:(st + 1) * 128, :].any():
                        nz = True
                        break
                if nz:
                    blk.append((r, st))
        ablocks.append(tuple(blk))

    in_maps = []
    for c in range(NCORES):
        xl = x[c * NLOC:(c + 1) * NLOC]
        m = dict(shared)
        m["xv"] = xl.astype(BF16)
        m["xt"] = np.ascontiguousarray(xl.T).astype(BF16)
        m["at"] = at_all[c * DPC:(c + 1) * DPC].astype(FP8)
        m["bt"] = bt_all[c * DPC:(c + 1) * DPC].astype(FP8)
        m["invd"] = inv_deg[c * NLOC:(c + 1) * NLOC].reshape(DPC, L)
        if use_mask:
            uml = umask[c * DPC:(c + 1) * DPC]   # (DPC, L)
            m["um"] = np.stack([uml * uml, uml], axis=1).astype(np.float32)
        in_maps.append(m)
    return in_maps, use_mask, tuple(halves), tuple(ablocks), perm


_last_results = None


def kernel(**inputs):
    global _last_results
    from concourse.bass_utils import run_bass_kernel_spmd

    in_maps, use_mask, halves, ablocks, perm = prep_inputs(**inputs)
    key = (use_mask, halves, ablocks)
    if key not in _cache:
        _cache[key] = _build_program(use_mask, halves, ablocks)
    nc = _cache[key]
    res = run_bass_kernel_spmd(nc, in_maps, core_ids=list(range(NCORES)))
    _last_results = res
    out_p = np.concatenate([res.results[c]["out"] for c in range(NCORES)], axis=0)
    out = np.empty_like(out_p)
    out[perm] = out_p
    return out


# revision 25
# speedup vs baseline: 1.0716x; 1.0716x over previous
"""Trainium2 Bass kernel for nn_DialogueGCNModel (DialogueGCN forward).

Strategy (data-parallel over dialogues, 4 dialogues per core):
  - Edges never cross dialogues (windowed construction), so the RGCN
    scatter/gather is reformulated as dense per-dialogue banded-adjacency
    matmuls: agg^T = (sum_r xr_r^T @ A_r^T) * (1/deg), with exact 0/1
    adjacency masks shipped as fp8 and the degree scaling applied in f32.
  - Everything on-device is dense PE matmuls (bf16/fp8 in, f32 accumulate),
    softmax/log-softmax in f32 on ACT/DVE.
  - Host does index preprocessing only: shard x, build per-dialogue 0/1
    adjacency masks from the edge lists, transpose/pack layouts, cast.
  - Emission is stage-major across the 4 dialogues so the PE never stalls
    on one dialogue's softmax chain; activations are function-major to
    avoid ACT LUT-table reloads; inputs move as a few large multi-dim-AP
    DMAs ordered by first use, and the PE runs dependency-free warm-up
    matmuls during the DMA lead-in to hold the HAM clock at 2.4 GHz.

kernel(**inputs) takes FULL inputs, runs 8-core SPMD via
bass_utils.run_bass_kernel_spmd, returns the FULL (8192, 7) f32 output.
"""

import numpy as np
import ml_dtypes

BF16 = ml_dtypes.bfloat16
FP8 = ml_dtypes.float8_e4m3

# Problem constants (hardcoded per contract)
B, L, D, H, R, NB, C = 32, 256, 1024, 128, 8, 30, 7
MEM = D + H            # 1152
N = B * L              # 8192
NCORES = 8
DPC = B // NCORES      # dialogues per core = 4
NLOC = DPC * L         # nodes per core = 1024
NT = NLOC // 128       # node tiles per core = 8
KT = D // 128          # contraction tiles over D = 8
MT = MEM // 128        # tiles over MEM = 9

_cache = {}


def _build_program(use_mask, halves, ablocks, stop_after=None):
    import concourse.bacc as bacc
    import concourse.tile as tile
    import concourse.mybir as mybir
    import concourse.bass as bass
    from concourse.masks import make_identity

    dt = mybir.dt
    f32, bf16, fp8 = dt.float32, dt.bfloat16, dt.float8e4
    AX = mybir.AxisListType.X
    AF = mybir.ActivationFunctionType
    OP = mybir.AluOpType

    nc = bacc.Bacc("TRN2", target_bir_lowering=False, debug=False,
                   num_devices=NCORES)

    dram = nc.dram_tensor
    xt_d = dram("xt", [D, NLOC], bf16, kind="ExternalInput")        # x^T [d, n]
    wrel_d = dram("wrel", [D, R * H], bf16, kind="ExternalInput")   # [d, r*H+h]
    wr1_d = dram("wr1", [D, H], bf16, kind="ExternalInput")
    at_d = dram("at", [DPC, R, L, L], fp8, kind="ExternalInput")    # A^T (0/1)
    bt_d = dram("bt", [DPC, L, L], fp8, kind="ExternalInput")       # B^T (0/1)
    invd_d = dram("invd", [DPC, L], f32, kind="ExternalInput")      # 1/deg
    w2_d = dram("w2", [2, H, H], bf16, kind="ExternalInput")        # rel2, root2
    wt_d = dram("wt", [MEM, MEM], bf16, kind="ExternalInput")
    wlin_d = dram("wlin", [MEM, H], bf16, kind="ExternalInput")
    wfc_d = dram("wfc", [H, C], bf16, kind="ExternalInput")
    bias_d = dram("bias", [128, 12], f32, kind="ExternalInput")
    bfc_d = dram("bfc", [1, C], bf16, kind="ExternalInput")
    if use_mask:
        um_d = dram("um", [DPC, 2, L], f32, kind="ExternalInput")   # um2, um
    out_d = dram("out", [NLOC, C], f32, kind="ExternalOutput")

    with tile.TileContext(nc) as tc:
        from contextlib import ExitStack
        with ExitStack() as ctx:
            consts = ctx.enter_context(tc.tile_pool(name="consts", bufs=1))
            big = ctx.enter_context(tc.tile_pool(name="big", bufs=1))
            work = ctx.enter_context(tc.tile_pool(name="work", bufs=4))
            ps = ctx.enter_context(tc.tile_pool(name="ps", bufs=5, space="PSUM"))
            pst = ctx.enter_context(tc.tile_pool(name="pst", bufs=2, space="PSUM"))

            dma_a = nc.sync.dma_start      # queue A: PE-critical operands
            dma_b = nc.scalar.dma_start    # queue B: everything else
            mm = nc.tensor.matmul

            # ---- persistent operand loads (one DMA per tensor) ----
            xt = consts.tile([128, KT, NLOC], bf16)
            dma_a(out=xt, in_=xt_d[:].rearrange("(k p) n -> p k n", p=128))
            wrel = consts.tile([128, KT, R * H], bf16)
            for h2 in range(2):
                dma_a(out=wrel[:, :, h2 * 512:(h2 + 1) * 512],
                      in_=wrel_d[:, h2 * 512:(h2 + 1) * 512]
                      .rearrange("(k p) n -> p k n", p=128))
            wr1 = consts.tile([128, KT, H], bf16)
            dma_a(out=wr1, in_=wr1_d[:].rearrange("(k p) n -> p k n", p=128))
            wt = consts.tile([128, MT, MEM], bf16)
            dma_a(out=wt, in_=wt_d[:].rearrange("(m p) n -> p m n", p=128))
            at = consts.tile([128, DPC, R, 2, L], fp8)
            dma_a(out=at,
                  in_=at_d[:].rearrange("d r (st p) t -> p d r st t", p=128))
            bt = consts.tile([128, DPC, 2, L], fp8)
            dma_a(out=bt, in_=bt_d[:].rearrange("d (st p) t -> p d st t", p=128))
            wlin = consts.tile([128, MT, H], bf16)
            dma_a(out=wlin, in_=wlin_d[:].rearrange("(m p) n -> p m n", p=128))
            w2 = consts.tile([128, 2, H], bf16)
            dma_b(out=w2, in_=w2_d[:].rearrange("j p h -> p j h"))
            wfc = consts.tile([128, C], bf16)
            dma_b(out=wfc, in_=wfc_d[:])
            bias = consts.tile([128, 12], f32)
            dma_b(out=bias, in_=bias_d[:])
            bfc = consts.tile([1, C], bf16)
            dma_b(out=bfc, in_=bfc_d[:])
            ones_row = consts.tile([1, 128], bf16)
            nc.vector.memset(ones_row, 1.0)
            ident = consts.tile([128, 128], bf16)
            make_identity(nc, ident)
            # keep the PE busy (HAM warm) during the input-DMA lead-in;
            # `warm` psum is never read.
            warm_in = consts.tile([128, 128], bf16)
            nc.vector.memset(warm_in, 0.0)
            pw = ctx.enter_context(tc.tile_pool(name="pw", bufs=1, space="PSUM"))
            warm = pw.tile([128, 128], f32, tag="warm")
            for _ in range(160):
                mm(warm, lhsT=warm_in, rhs=warm_in, start=True, stop=True,
                   skip_group_check=True)

            def bcast(dst, src_ap):
                bc = bass.AP(tensor=src_ap.tensor, offset=src_ap.offset,
                             ap=[[0, 128]] + list(src_ap.ap))
                nc.gpsimd.dma_start(out=dst, in_=bc)

            invd = consts.tile([128, DPC, L], f32)
            bcast(invd, invd_d[:])
            if use_mask:
                um = consts.tile([128, DPC, 2, L], f32)
                bcast(um, um_d[:])

            # ---- stage 1: xr[n, (r,h)] = x @ w_rel (all relations) ----
            xr = consts.tile([128, NT, R * H], bf16)
            for h2, i in sorted(
                    (h2, i) for i in range(NT) for h2 in halves[i]):
                p = ps.tile([128, 512], f32, tag="mm")
                for k in range(KT):
                    mm(p, lhsT=xt[:, k, i * 128:(i + 1) * 128],
                       rhs=wrel[:, k, h2 * 512:(h2 + 1) * 512],
                       start=(k == 0), stop=(k == KT - 1))
                nc.vector.tensor_copy(xr[:, i, h2 * 512:(h2 + 1) * 512], p)

            if stop_after == "xr":
                return _finish(nc)

            out1T = consts.tile([128, DPC, L], bf16)   # [h, dlg, n]
            out1 = consts.tile([128, NT, H], bf16)     # [n, h]
            out2T = consts.tile([128, DPC, L], bf16)
            out2 = consts.tile([128, NT, H], bf16)
            hidT = consts.tile([128, DPC, L], bf16)

            # ---- stage 2: out1^T = (sum_r xr_r^T A_r^T)*invd + root^T + b1
            for d in range(DPC):
                n0 = d * L
                pa = ps.tile([128, 512], f32, tag="mm")
                blocks = ablocks[d]
                for bi, (r, st) in enumerate(blocks):
                    mm(pa[:, :L], lhsT=xr[:, 2 * d + st, r * H:(r + 1) * H],
                       rhs=at[:, d, r, st, :], start=(bi == 0),
                       stop=(bi == len(blocks) - 1))
                agg = work.tile([128, L], f32, tag="agg")
                nc.vector.tensor_mul(agg, pa[:, :L], invd[:, d, :])
                pr = ps.tile([128, 512], f32, tag="mm")
                for k in range(KT):
                    mm(pr[:, :L], lhsT=wr1[:, k, :], rhs=xt[:, k, n0:n0 + L],
                       start=(k == 0), stop=(k == KT - 1))
                nc.vector.scalar_tensor_tensor(
                    out=out1T[:, d, :], in0=pr[:, :L], scalar=bias[:, 0:1],
                    in1=agg, op0=OP.add, op1=OP.add)
            for d in range(DPC):
                for st in range(2):
                    tp = pst.tile([128, 128], bf16, tag="tr")
                    nc.tensor.transpose(tp, out1T[:, d, st * 128:(st + 1) * 128], ident)
                    nc.vector.tensor_copy(out1[:, 2 * d + st, :], tp)

            # ---- stage 3: GraphConv layer 2 ----
            nbTs = []
            for d in range(DPC):
                p2 = ps.tile([128, 512], f32, tag="mm")
                for st in range(2):
                    mm(p2[:, :L], lhsT=out1[:, 2 * d + st, :], rhs=bt[:, d, st, :],
                       start=(st == 0), stop=(st == 1))
                nbT = work.tile([128, L], bf16, tag="nbT")
                nc.vector.tensor_copy(nbT, p2[:, :L])
                nbTs.append(nbT)
            for d in range(DPC):
                p3 = ps.tile([128, 512], f32, tag="mm")
                mm(p3[:, :L], lhsT=w2[:, 0, :], rhs=nbTs[d], start=True, stop=False)
                mm(p3[:, :L], lhsT=w2[:, 1, :], rhs=out1T[:, d, :],
                   start=False, stop=True)
                nc.scalar.activation(out2T[:, d, :], p3[:, :L], AF.Identity,
                                     bias=bias[:, 1:2])
            for d in range(DPC):
                for st in range(2):
                    tp = pst.tile([128, 128], bf16, tag="tr")
                    nc.tensor.transpose(tp, out2T[:, d, st * 128:(st + 1) * 128], ident)
                    nc.vector.tensor_copy(out2[:, 2 * d + st, :], tp)

            if stop_after == "rgcn":
                return _finish(nc)

            # M^T / M tile accessors over MEM
            def rhs_MT(mt_i, d):
                if mt_i < KT:
                    return xt[:, mt_i, d * L:(d + 1) * L]
                return out2T[:, d, :]

            # ---- stage 5: Xc^T = w_t^T M^T + b_t ----
            XcTs = []
            for d in range(DPC):
                XcT = big.tile([128, MT, L], bf16, tag=f"XcT{d}")
                XcTs.append(XcT)
                for n2 in range(MT):
                    p4 = ps.tile([128, 512], f32, tag="mm")
                    for m in range(MT):
                        mm(p4[:, :L], lhsT=wt[:, m, n2 * 128:(n2 + 1) * 128],
                           rhs=rhs_MT(m, d), start=(m == 0), stop=(m == MT - 1))
                    nc.scalar.activation(XcT[:, n2, :], p4[:, :L], AF.Identity,
                                         bias=bias[:, 2 + n2:3 + n2])

            if stop_after == "xc":
                return _finish(nc)

            # ---- stage 6: scores -> tanh -> masked softmax -> alpha^T ----
            # function-major: all tanh, then all exp (one ACT table load each)
            zs, nmxs = {}, {}
            for d in range(DPC):
                for tt in range(2):
                    p5 = ps.tile([128, 512], f32, tag="mm")
                    for n2 in range(MT):
                        mm(p5[:, :L], lhsT=XcTs[d][:, n2, tt * 128:(tt + 1) * 128],
                           rhs=rhs_MT(n2, d), start=(n2 == 0), stop=(n2 == MT - 1))
                    z = big.tile([128, L], f32, tag=f"z{d}{tt}")
                    if use_mask:
                        nc.vector.tensor_mul(z, p5[:, :L], um[:, d, 0, :])
                        nc.scalar.activation(z, z, AF.Tanh)
                    else:
                        nc.scalar.activation(z, p5[:, :L], AF.Tanh)
                    nmx = work.tile([128, 1], f32, tag="nmx")
                    nc.vector.reduce_max(out=nmx, in_=z, axis=AX, negate=True)
                    zs[(d, tt)] = z
                    nmxs[(d, tt)] = nmx
            alfs = {}
            for d in range(DPC):
                for tt in range(2):
                    z, nmx = zs[(d, tt)], nmxs[(d, tt)]
                    ssum = work.tile([128, 1], f32, tag="ssum")
                    nc.scalar.activation(z, z, AF.Exp, bias=nmx, accum_out=ssum)
                    if use_mask:
                        nc.vector.tensor_mul(z, z, um[:, d, 1, :])
                        nc.vector.reduce_sum(out=ssum, in_=z, axis=AX)
                    rinv = work.tile([128, 1], f32, tag="rinv")
                    nc.vector.reciprocal(rinv, ssum)
                    alf = big.tile([128, L], bf16, tag=f"alf{d}{tt}")
                    nc.vector.tensor_scalar_mul(alf, z, rinv)
                    alfs[(d, tt)] = alf
            # ---- stage 6.5: G = M @ w_lin (att@w_lin reassociated; att is
            # never materialized: hidden = relu(alpha @ G + b_lin))
            Gs = {}
            for d in range(DPC):
                for st in range(2):
                    pg = ps.tile([128, 512], f32, tag="mm")
                    for m in range(MT):
                        mm(pg[:, :H],
                           lhsT=rhs_MT(m, d)[:, st * 128:(st + 1) * 128],
                           rhs=wlin[:, m, :], start=(m == 0), stop=(m == MT - 1))
                    G = big.tile([128, H], bf16, tag=f"G{d}{st}")
                    if st == 0:
                        nc.vector.tensor_copy(G, pg[:, :H])
                    else:
                        nc.scalar.copy(G, pg[:, :H])
                    Gs[(d, st)] = G

            alphaTs = []
            for d in range(DPC):
                alphaT = big.tile([128, 2, L], bf16, tag=f"alphaT{d}")
                alphaTs.append(alphaT)
                for tt in range(2):
                    for st in range(2):
                        tp = pst.tile([128, 128], bf16, tag="tr")
                        nc.tensor.transpose(
                            tp, alfs[(d, tt)][:, st * 128:(st + 1) * 128], ident)
                        nc.vector.tensor_copy(
                            alphaT[:, st, tt * 128:(tt + 1) * 128], tp)

            if stop_after == "scores":
                return _finish(nc)

            # ---- stage 7: hidden^T = relu(G^T @ alpha^T + b_lin) ----
            for d in range(DPC):
                p7 = ps.tile([128, 512], f32, tag="mm")
                for st in range(2):
                    mm(p7[:, :L], lhsT=Gs[(d, st)], rhs=alphaTs[d][:, st, :],
                       start=(st == 0), stop=(st == 1))
                nc.scalar.activation(hidT[:, d, :], p7[:, :L], AF.Relu,
                                     bias=bias[:, 11:12])

            if stop_after == "att":
                return _finish(nc)

            # ---- stage 8: logits + log_softmax (function-major) ----
            o_all = consts.tile([128, DPC, 2, 8], f32)
            nm7s, s7s = {}, {}
            for d in range(DPC):
                for tt in range(2):
                    p8 = ps.tile([128, 512], f32, tag="mm")
                    mm(p8[:, :C], lhsT=hidT[:, d, tt * 128:(tt + 1) * 128],
                       rhs=wfc, start=True, stop=False)
                    mm(p8[:, :C], lhsT=ones_row, rhs=bfc, start=False, stop=True)
                    nm7 = work.tile([128, 1], f32, tag=f"nm7_{d}{tt}")
                    nc.vector.reduce_max(out=nm7, in_=p8[:, :C], axis=AX, negate=True)
                    e7 = work.tile([128, 8], f32, tag="e7")
                    s7 = work.tile([128, 1], f32, tag=f"s7_{d}{tt}")
                    nc.scalar.activation(e7[:, :C], p8[:, :C], AF.Exp,
                                         bias=nm7, accum_out=s7)
                    nc.vector.tensor_scalar_add(o_all[:, d, tt, :C], p8[:, :C], nm7)
                    nm7s[(d, tt)], s7s[(d, tt)] = nm7, s7
            for d in range(DPC):
                for tt in range(2):
                    nm7, s7 = nm7s[(d, tt)], s7s[(d, tt)]
                    ls7 = work.tile([128, 1], f32, tag="ls7")
                    nc.scalar.activation(ls7, s7, AF.Ln)
                    nc.vector.tensor_scalar(
                        out=o_all[:, d, tt, :C], in0=o_all[:, d, tt, :C],
                        scalar1=ls7, scalar2=None, op0=OP.subtract)
            dma_a(out=out_d[:].rearrange("(d tt p) c -> p d tt c", d=DPC, tt=2),
                  in_=o_all[:, :, :, 0:C])

    return _finish(nc)


def _finish(nc):
    nc.compile()
    return nc


def prep_inputs(x, edge_src, edge_dst, edge_type, umask, basis, comp,
                w_root1, b1, w_rel2, b_rel2, w_root2, w_t, b_t,
                w_lin, b_lin, w_fc, b_fc):
    """Host-side sharding / layout prep.

    Returns (in_maps, use_mask, halves, ablocks, perm).
    Nodes are permuted within each dialogue so same-speaker nodes are
    contiguous; then each 128-node tile only needs the relation-half
    matching its speaker(s), and all-zero adjacency blocks are skipped.
    """
    x = np.asarray(x, np.float32)
    src = np.asarray(edge_src, np.int64)
    dst = np.asarray(edge_dst, np.int64)
    ety = np.asarray(edge_type, np.int64)
    umask = np.asarray(umask, np.float32)
    basis = np.asarray(basis, np.float32)
    comp = np.asarray(comp, np.float32)

    # dialogue-locality of edges (guaranteed by the windowed construction)
    g_s = src // L
    assert np.array_equal(g_s, dst // L), "edges must stay within a dialogue"

    # infer per-node speaker from edge types (etype = s_src*4 + s_dst*2 + dir);
    # fall back to identity permutation if inconsistent.
    spk_s = ety // 4
    spk_d = (ety // 2) % 2
    spk = -np.ones(N, np.int64)
    spk[src] = spk_s
    ok = bool(np.all(spk[src] == spk_s)) and bool(np.all(spk[dst] == spk_d)) \
        and bool(np.all(spk >= 0))
    if ok:
        perm = np.empty(N, np.int64)
        for g in range(B):
            sl = slice(g * L, (g + 1) * L)
            perm[sl] = g * L + np.argsort(spk[sl], kind="stable")
    else:
        perm = np.arange(N, dtype=np.int64)
    inv = np.empty(N, np.int64)
    inv[perm] = np.arange(N)

    x = x[perm]
    src = inv[src]
    dst = inv[dst]
    umask = umask.reshape(N)[perm].reshape(B, L)
    spk_p = spk[perm] if ok else spk

    # w_rel[r] = sum_b comp[r,b] basis[b]  -> layout [d, r*H+h]
    w_rel = np.einsum('rb,bdh->rdh', comp, basis)
    wrel_layout = np.ascontiguousarray(
        w_rel.transpose(1, 0, 2).reshape(D, R * H)).astype(BF16)

    deg = np.bincount(dst, minlength=N).astype(np.float64)
    inv_deg = np.where(deg > 0, 1.0 / np.maximum(deg, 1), 0.0).astype(np.float32)

    g_s = src // L
    at_all = np.zeros((B, R, L, L), np.float32)   # [dlg, r, src, dst] 0/1
    ls, ld = src % L, dst % L
    np.add.at(at_all, (g_s, ety, ls, ld), 1.0)
    bt_all = np.zeros((B, L, L), np.float32)
    np.add.at(bt_all, (g_s, ls, ld), 1.0)

    use_mask = not bool(np.all(umask == 1.0))

    bias_pack = np.zeros((128, 12), np.float32)
    bias_pack[:, 0] = np.asarray(b1, np.float32)
    bias_pack[:, 1] = np.asarray(b_rel2, np.float32)
    bias_pack[:, 2:11] = np.asarray(b_t, np.float32).reshape(9, 128).T
    bias_pack[:, 11] = np.asarray(b_lin, np.float32)

    shared = {
        "wrel": wrel_layout,
        "wr1": np.asarray(w_root1, np.float32).astype(BF16),
        "w2": np.stack([np.asarray(w_rel2, np.float32),
                        np.asarray(w_root2, np.float32)]).astype(BF16),
        "wt": np.asarray(w_t, np.float32).astype(BF16),
        "wlin": np.asarray(w_lin, np.float32).astype(BF16),
        "wfc": np.asarray(w_fc, np.float32).astype(BF16),
        "bias": bias_pack,
        "bfc": np.asarray(b_fc, np.float32).reshape(1, C).astype(BF16),
    }

    # per-core tile structure: which relation-halves each node-tile needs,
    # and which (r, st) adjacency blocks are nonzero per dialogue.
    # NOTE: the program structure must be IDENTICAL across cores (one SPMD
    # NEFF), so take the union over cores per (tile, dialogue) position.
    halves = []
    for i in range(NT):
        hs = set()
        for c in range(NCORES):
            nodes = slice(c * NLOC + i * 128, c * NLOC + (i + 1) * 128)
            if ok:
                hs |= set(np.unique(spk_p[nodes]).tolist())
            else:
                hs |= {0, 1}
        halves.append(tuple(sorted(hs)))
    ablocks = []
    for d in range(DPC):
        blk = []
        for r in range(R):
            for st in range(2):
                nz = False
                for c in range(NCORES):
                    g = c * DPC + d
                    if at_all[g, r, st * 128:(st + 1) * 128, :].any():
                        nz = True
                        break
                if nz:
                    blk.append((r, st))
        ablocks.append(tuple(blk))

    in_maps = []
    for c in range(NCORES):
        xl = x[c * NLOC:(c + 1) * NLOC]
        m = dict(shared)
        m["xt"] = np.ascontiguousarray(xl.T).astype(BF16)
        m["at"] = at_all[c * DPC:(c + 1) * DPC].astype(FP8)
        m["bt"] = bt_all[c * DPC:(c + 1) * DPC].astype(FP8)
        m["invd"] = inv_deg[c * NLOC:(c + 1) * NLOC].reshape(DPC, L)
        if use_mask:
            uml = umask[c * DPC:(c + 1) * DPC]   # (DPC, L)
            m["um"] = np.stack([uml * uml, uml], axis=1).astype(np.float32)
        in_maps.append(m)
    return in_maps, use_mask, tuple(halves), tuple(ablocks), perm


_last_results = None


def kernel(**inputs):
    global _last_results
    from concourse.bass_utils import run_bass_kernel_spmd

    in_maps, use_mask, halves, ablocks, perm = prep_inputs(**inputs)
    key = (use_mask, halves, ablocks)
    if key not in _cache:
        _cache[key] = _build_program(use_mask, halves, ablocks)
    nc = _cache[key]
    res = run_bass_kernel_spmd(nc, in_maps, core_ids=list(range(NCORES)))
    _last_results = res
    out_p = np.concatenate([res.results[c]["out"] for c in range(NCORES)], axis=0)
    out = np.empty_like(out_p)
    out[perm] = out_p
    return out
